# revision 58
# baseline (speedup 1.0000x reference)
"""Causal single-head attention (B=4, S=4096, E=1024, D=128) on 8 TRN2 cores.

Sharding: core c = (batch b = c//2, key-parity p = c%2). Each core processes
ALL 4096 queries of its batch against the 2048 keys in the even (p=0) or odd
(p=1) 128-token key blocks, producing UNNORMALIZED partial attention
AV^T [D, S] and partial softmax denominators den [1, S]. The host combines
the two parity cores per batch: out = ((AV_e + AV_o) / (den_e + den_o))^T.
This makes every query group g attend a uniform prefix of 2(g+1) pool key
blocks on every core (72 blocks total vs 104 for the half-query sharding),
halves the K/V projection (no duplication across the pair), and needs no
collectives and no rectangle masking.

Within a pool prefix, the last 2 blocks straddle the causal diagonal; their
[128, 1024] staircase mask is per-core DATA (host-computed, parity-dependent)
added once per q-group (DVE for early groups, PE ident-matmul for late ones).
Everything else is unmasked. The softmax denominator is taken off the PE
entirely: exp pairs accumulate on DVE (bf16) and the (otherwise idle) Pool
engine partition-reduces the accumulator; only the last group folds its den
on PE to shorten the drain tail. Projection matmuls are emitted as a thunk
stream that the attention loop pulls as PE fillers while ScalarE (the
attention-phase bottleneck at ~1038ns/pair vs ~852ns of PE work) computes
exps.

All inputs are host-cast to bf16 (halves HBM traffic; TensorE runs bf16 at
1 cycle/row). x arrives pre-transposed per batch as xT [E, S]; K/V consume
parity-strided token slices via 3-dim moving APs, V is projected directly in
[s, d] layout (no PE transposes), and the AV partials are stored [d, q] and
transposed on the host. Scores are built transposed ([k, q]) so exp fuses on
ScalarE over [128, 1024] block-pairs and the AV/den matmuls consume exp
output directly. Softmax skips max-subtraction (score*scale stays ~[-5, 5]
for randn inputs)."""

import sys

if "/opt/trn_rl_repo" not in sys.path:
    sys.path.insert(0, "/opt/trn_rl_repo")

import numpy as np

B, S, E, D = 4, 4096, 1024, 128
SCALE = 1.0 / 32.0  # 1/sqrt(E)
NEG = -1.0e9
P = 128
QW = 512  # query group width
ECH = E // P  # 8 e-chunks
NQG = S // QW  # 8 query groups
NT = 4  # x chunks / kv pool groups (1024 tokens each)
XW = 1024


KNOBS = {
    "mask_pe_min_g": 6,   # groups >= this use PE ident-matmul for the mask
    "den_pe_min_g": 7,    # groups >= this compute den on PE instead of Pool
    "pull_diag": 1400,     # filler budget (ns) while the diag exp runs
    "pull_pair": 190,
    "pull_pair_late": 190,     # filler budget (ns) per non-diag pair
    "expt_bufs": 6,
    "acc_bufs": 4,
}


def _build():
    parity = 0  # odd-parity cores get host-swapped xt columns (see kernel())
    import concourse.bass as bass  # noqa: F401
    import concourse.mybir as mybir
    import concourse.tile as tile
    from concourse import bacc, bass_isa
    from concourse.masks import make_identity

    f32 = mybir.dt.float32
    bf16 = mybir.dt.bfloat16

    nc = bacc.Bacc(
        "TRN2",
        target_bir_lowering=False,
        debug=False,
        enable_asserts=False,
        num_devices=8,
    )

    xt_d = nc.dram_tensor("xt", [E, S], bf16, kind="ExternalInput").ap()
    wq_d = nc.dram_tensor("wq", [P, ECH * D], bf16, kind="ExternalInput").ap()
    wk_d = nc.dram_tensor("wk", [P, ECH * D], bf16, kind="ExternalInput").ap()
    wv_d = nc.dram_tensor("wv", [P, ECH * D], bf16, kind="ExternalInput").ap()
    mk_d = nc.dram_tensor("mk", [P, 2 * QW], bf16, kind="ExternalInput").ap()
    av_d = nc.dram_tensor("av", [P, S], bf16, kind="ExternalOutput").ap()
    den_d = nc.dram_tensor("den", [1, S], f32, kind="ExternalOutput").ap()

    with tile.TileContext(nc) as tc:
        from contextlib import ExitStack

        with ExitStack() as ctx:
            consts = ctx.enter_context(tc.tile_pool(name="consts", bufs=1))
            xraw_p = ctx.enter_context(tc.tile_pool(name="xraw", bufs=32))
            kv_p = ctx.enter_context(tc.tile_pool(name="kv", bufs=1))
            expt_p = ctx.enter_context(tc.tile_pool(name="expt", bufs=KNOBS["expt_bufs"]))
            acc_p = ctx.enter_context(tc.tile_pool(name="acc", bufs=KNOBS["acc_bufs"]))
            avsb_p = ctx.enter_context(tc.tile_pool(name="avsb", bufs=2))
            ps_sc = ctx.enter_context(tc.tile_pool(name="ps_sc", bufs=2, space="PSUM"))
            ps_proj = ctx.enter_context(
                tc.tile_pool(name="ps_proj", bufs=2, space="PSUM")
            )
            ps_av = ctx.enter_context(tc.tile_pool(name="ps_av", bufs=2, space="PSUM"))
            red_p = ctx.enter_context(tc.tile_pool(name="red", bufs=2))

            # ---- weights / masks / constants (wk first: K-proj starts
            # earliest; ec0 chunk split off so the first matmul's DMA chain
            # is overhead-dominated, not transfer-dominated) ----
            wk_sb = consts.tile([P, ECH * D], bf16, tag="w_wk", name="wsb_wk")
            nc.scalar.dma_start(wk_sb[:, 0:D], wk_d[:, 0:D])
            nc.scalar.dma_start(wk_sb[:, D:], wk_d[:, D:])

            xr = {}

            def load_chunk(t):
                for ec in range(ECH):
                    xh = xraw_p.tile([P, XW], bf16, tag="xraw", name=f"xr{t}_{ec}")
                    nc.sync.dma_start(
                        xh[:],
                        xt_d[ec * P : (ec + 1) * P, t * XW : (t + 1) * XW],
                    )
                    xr[(t, ec)] = xh

            load_chunk(0)

            wv_sb = consts.tile([P, ECH * D], bf16, tag="w_wv", name="wsb_wv")
            nc.scalar.dma_start(wv_sb[:], wv_d[:])
            wq_sb = consts.tile([P, ECH * D], bf16, tag="w_wq", name="wsb_wq")
            nc.scalar.dma_start(wq_sb[:], wq_d[:])
            mk_sb = consts.tile([P, 2 * QW], bf16, tag="mk")
            nc.scalar.dma_start(mk_sb[:], mk_d[:])
            ones = consts.tile([P, 1], bf16, tag="ones")
            nc.gpsimd.memset(ones[:], 1.0)
            ident = consts.tile([P, P], bf16, tag="ident")
            make_identity(nc, ident[:])
            densb = consts.tile([1, S], f32, tag="densb")

            for t in range(1, NT):
                load_chunk(t)

            # per-pool-group projected tiles
            kt_g = [kv_p.tile([P, QW], bf16, tag=f"kt{t}", name=f"kt{t}") for t in range(NT)]
            v_g = [kv_p.tile([P, QW], bf16, tag=f"v{t}", name=f"v{t}") for t in range(NT)]
            qt_g = [kv_p.tile([P, QW], bf16, tag=f"qt{g}", name=f"qt{g}") for g in range(NQG)]

            # ---- projection emission stream ----
            # Each thunk is (phase_t, est_pe_ns, fn). The attention loop pulls
            # thunks as PE fillers while ScalarE computes exps (attention is
            # ACT-bound at ~1038ns/pair vs ~852ns of PE work), and drains all
            # thunks of phase t before attention needs them.

            def k_unit(t):
                pk = ps_proj.tile([P, QW], f32, tag="proj")
                for ec in range(ECH):
                    rhs3 = xr[(t, ec)][:].rearrange(
                        "p (f two h) -> p f two h", f=4, two=2
                    )[:, :, parity, :]
                    yield 213, lambda ec=ec, rhs3=rhs3: nc.tensor.matmul(
                        pk[:],
                        wk_sb[:, ec * D : (ec + 1) * D],
                        rhs3,
                        start=(ec == 0),
                        stop=(ec == ECH - 1),
                    )
                yield 0, lambda: nc.vector.tensor_copy(kt_g[t][:], pk[:])

            def v_unit(t):
                # ec-outer so each arriving x-chunk immediately unlocks work.
                # ONE psum accumulation group for the whole bank: start marks
                # the full 2KB zero region, so each mloc's first write
                # overwrites (pending-zero) and later ecs accumulate.
                pv = ps_proj.tile([P, QW], f32, tag="proj")
                for ec in range(ECH):
                    for mloc in range(4):
                        off = mloc * 256 + parity * P
                        yield 53, lambda mloc=mloc, off=off, ec=ec: nc.tensor.matmul(
                            pv[:, mloc * P : (mloc + 1) * P],
                            xr[(t, ec)][:, off : off + P],
                            wv_sb[:, ec * D : (ec + 1) * D],
                            start=(ec == 0 and mloc == 0),
                            stop=(ec == ECH - 1 and mloc == 3),
                        )
                yield 0, lambda: nc.vector.tensor_copy(v_g[t][:], pv[:])

            def q_unit(g):
                t, half = g // 2, (g % 2) * QW
                pq = ps_proj.tile([P, QW], f32, tag="proj")
                for ec in range(ECH):
                    yield 213, lambda ec=ec: nc.tensor.matmul(
                        pq[:],
                        wq_sb[:, ec * D : (ec + 1) * D],
                        xr[(t, ec)][:, half : half + QW],
                        start=(ec == 0),
                        stop=(ec == ECH - 1),
                    )
                yield 0, lambda: nc.vector.tensor_copy(qt_g[g][:], pq[:])

            from collections import deque

            # Unit labels: group g's attention may start once all units with
            # label <= g are emitted. K/V/Q(2t) of phase t carry label 2t;
            # Q(2t+1) carries label 2t+1 so it fills att(2t)'s ACT-bound
            # slack instead of draining serially.
            proj_q = deque()
            for t in range(NT):
                if t == 0:
                    # interleave phase-0 K, V, Q0, Q1 per e-chunk so each
                    # x-chunk DMA (712ns apart) unlocks ~850ns of PE work.
                    # K/V use the two ps_proj bufs; Q0/Q1 share a [P,2QW]
                    # "sc"-tagged psum tile (two independent 2KB zero
                    # regions) since the sc pool is idle during startup.
                    kl = list(k_unit(0))  # 8 matmuls + copy
                    vl = list(v_unit(0))  # 32 matmuls + copy
                    pq01 = ps_sc.tile([P, 2 * QW], f32, tag="sc")

                    def q01_mm(ec, half):
                        return lambda: nc.tensor.matmul(
                            pq01[:, half * QW : (half + 1) * QW],
                            wq_sb[:, ec * D : (ec + 1) * D],
                            xr[(0, ec)][:, half * QW : (half + 1) * QW],
                            start=(ec == 0),
                            stop=(ec == ECH - 1),
                        )

                    for ec in range(ECH):
                        proj_q.append((0,) + kl[ec])
                        for j in range(4):
                            proj_q.append((0,) + vl[4 * ec + j])
                        proj_q.append((0, 213, q01_mm(ec, 0)))
                        proj_q.append((0, 213, q01_mm(ec, 1)))
                    proj_q.append((0,) + kl[8])
                    proj_q.append((0,) + vl[32])
                    proj_q.append(
                        (0, 0, lambda: nc.vector.tensor_copy(qt_g[0][:], pq01[:, 0:QW]))
                    )
                    proj_q.append(
                        (
                            1,
                            0,
                            lambda: nc.vector.tensor_copy(
                                qt_g[1][:], pq01[:, QW : 2 * QW]
                            ),
                        )
                    )
                else:
                    # Q(2t) carries label 2t-0.5 (needed at att(2t) start);
                    # K(t)/V(t) carry 2t: att(2t) opens with pair g-1 (old
                    # kt), so K/V drain as fillers before its diag pair
                    for th in q_unit(2 * t):
                        proj_q.append((2 * t - 0.5,) + th)
                    for th in k_unit(t):
                        proj_q.append((2 * t,) + th)
                    for th in v_unit(t):
                        proj_q.append((2 * t,) + th)
                    for th in q_unit(2 * t + 1):
                        proj_q.append((2 * t + 0.5,) + th)

            def drain_label(lbl):
                while proj_q and proj_q[0][0] <= lbl:
                    _, _, fn = proj_q.popleft()
                    fn()

            def pull(ns, maxlbl):
                # fillers must not outrun the x-chunk DMAs: a PE instruction
                # waiting on a DMA would block the whole in-order PE queue
                while ns > 0 and proj_q and proj_q[0][0] <= maxlbl:
                    _, est, fn = proj_q.popleft()
                    fn()
                    ns -= max(est, 1)

            # ---- attention; diagonal pair first so its mask-add + exp
            # latency hides under the remaining pairs' scores ----
            for t in range(NT):
                for g in (2 * t, 2 * t + 1):
                    drain_label(g - 0.5)  # qt(g); K/V may drain at the diag
                    # late groups run with the proj stream exhausted (PE has
                    # slack): apply the diag mask via PE ident-matmul into
                    # PSUM instead of a DVE add (shorter diag-exp chain), and
                    # for the last group compute den on PE from acc so the
                    # tail skips the Pool-reduce chain.
                    mask_on_pe = g >= KNOBS["mask_pe_min_g"]
                    den_from_pe = g >= KNOBS["den_pe_min_g"]
                    pav = ps_av.tile([P, QW], f32, tag="av")
                    acc = acc_p.tile([P, 2 * QW], bf16, tag="acc")
                    order = [g - 1, g] + list(range(g - 1)) if g else [0]
                    for i, pr in enumerate(order):
                        if pr == g:
                            drain_label(g)  # kt/v for the diagonal pair
                        psc = ps_sc.tile([P, 2 * QW], f32, tag="sc")
                        diag = pr == g
                        for half in range(2):
                            m = 2 * pr + half  # pool block index
                            tk, ck = m // 4, (m % 4) * P
                            nc.tensor.matmul(
                                psc[:, half * QW : (half + 1) * QW],
                                kt_g[tk][:, ck : ck + P],
                                qt_g[g][:],
                                start=True,
                                stop=not (diag and mask_on_pe),
                            )
                            if diag and mask_on_pe:
                                nc.tensor.matmul(
                                    psc[:, half * QW : (half + 1) * QW],
                                    ident[:],
                                    mk_sb[:, half * QW : (half + 1) * QW],
                                    start=False,
                                    stop=True,
                                )
                        if diag and not mask_on_pe:
                            nc.vector.tensor_add(psc[:], psc[:], mk_sb[:])
                        if i == 0:
                            et = acc  # exp writes the accumulator directly
                        else:
                            et = expt_p.tile([P, 2 * QW], bf16, tag="expt")
                        nc.scalar.activation(
                            et[:],
                            psc[:],
                            mybir.ActivationFunctionType.Exp,
                            bias=0.0,
                            scale=SCALE,
                        )
                        # PE fillers run while ScalarE computes this exp
                        pull(KNOBS["pull_diag"] if i == 0 else (KNOBS["pull_pair_late"] if g >= 4 else KNOBS["pull_pair"]), g + 1)
                        last = i == g
                        if den_from_pe and last:
                            # den over pairs 0..g-1 from acc now (off the tail
                            # chain); the final pair's et accumulates below
                            pden = ps_proj.tile([1, QW], f32, tag="proj")
                            for half in range(2):
                                nc.tensor.matmul(
                                    pden[:],
                                    ones[:],
                                    acc[:, half * QW : (half + 1) * QW],
                                    start=(half == 0),
                                    stop=False,
                                )
                        for half in range(2):
                            m = 2 * pr + half
                            tk, ck = m // 4, (m % 4) * P
                            nc.tensor.matmul(
                                pav[:],
                                v_g[tk][:, ck : ck + P],
                                et[:, half * QW : (half + 1) * QW],
                                start=(i == 0 and half == 0),
                                stop=(i == g and half == 1),
                            )
                        if den_from_pe and last:
                            for half in range(2):
                                nc.tensor.matmul(
                                    pden[:],
                                    ones[:],
                                    et[:, half * QW : (half + 1) * QW],
                                    start=False,
                                    stop=(half == 1),
                                )
                        elif i > 0:
                            nc.vector.tensor_add(acc[:], acc[:], et[:])
                    if den_from_pe:
                        nc.vector.tensor_copy(
                            densb[:, g * QW : (g + 1) * QW], pden[:]
                        )
                    else:
                        # den: partition-reduce the exp accumulator on the
                        # (idle) Pool engine, fold halves on DVE — no PE/PSUM
                        red = red_p.tile([P, 2 * QW], f32, tag="red")
                        nc.gpsimd.partition_all_reduce(
                            red[:], acc[:], channels=P,
                            reduce_op=bass_isa.ReduceOp.add,
                        )
                        nc.vector.tensor_add(
                            densb[:, g * QW : (g + 1) * QW],
                            red[0:1, 0:QW],
                            red[0:1, QW : 2 * QW],
                        )
                    nc.sync.dma_start(
                        den_d[:, g * QW : (g + 1) * QW],
                        densb[:, g * QW : (g + 1) * QW],
                    )
                    avsb = avsb_p.tile([P, QW], bf16, tag="avsb")
                    nc.vector.tensor_copy(avsb[:], pav[:])
                    nc.sync.dma_start(av_d[:, g * QW : (g + 1) * QW], avsb[:])

    nc.compile()
    return nc


_NC = None
LAST_RESULTS = None


def _masks(parity):
    """Mask for the diagonal block-pair of each query group, against the
    core's (possibly half-swapped) local query order."""
    import ml_dtypes

    mk = np.zeros((P, 2 * QW), dtype=np.float32)
    k = np.arange(P)[:, None]
    ql = np.arange(QW)[None, :]
    if parity:
        # local query ql maps to abs in-group offset with 128-halves of each
        # 256-span swapped
        sig = (ql // 256) * 256 + (1 - (ql % 256) // P) * P + (ql % P)
    else:
        sig = ql
    for dm in range(2):
        allowed = sig >= (k + 256 * dm + P * parity)
        mk[:, dm * QW : (dm + 1) * QW] = np.where(allowed, 0.0, NEG)
    return np.ascontiguousarray(mk.astype(ml_dtypes.bfloat16))


def kernel(x, WQ, WK, WV):
    import os

    import ml_dtypes

    from concourse import bass_utils

    global LAST_RESULTS, _NC
    bf = ml_dtypes.bfloat16
    x = np.asarray(x, dtype=np.float32)

    def prep_w(W):
        # [E, D] -> [P, ECH*D] with chunk ec at columns [ec*D, (ec+1)*D)
        W = np.asarray(W, dtype=np.float32)
        return np.ascontiguousarray(
            W.reshape(ECH, P, D).transpose(1, 0, 2).reshape(P, ECH * D)
        ).astype(bf)

    wq_h, wk_h, wv_h = prep_w(WQ), prep_w(WK), prep_w(WV)
    # column permutation: swap the two 128-halves of every 256-token span so
    # odd-parity cores see their key blocks in "slot 0" of each span
    swap = (
        (np.arange(S) // 256) * 256 + (1 - (np.arange(S) % 256) // P) * P
        + (np.arange(S) % P)
    )
    xt_nat = [np.ascontiguousarray(x[b].T.astype(bf)) for b in range(B)]
    xt_swp = [np.ascontiguousarray(xb[:, swap]) for xb in xt_nat]
    mk_all = [_masks(p) for p in range(2)]

    if _NC is None:
        _NC = _build()
    nc = _NC

    in_maps = []
    for c in range(8):
        b, p = c >> 1, c & 1
        in_maps.append(
            {
                "xt": xt_nat[b] if p == 0 else xt_swp[b],
                "wq": wq_h,
                "wk": wk_h,
                "wv": wv_h,
                "mk": mk_all[p],
            }
        )

    trace = os.environ.get("KERNEL_TRACE") == "1"
    res = bass_utils.run_bass_kernel_spmd(
        nc, in_maps, core_ids=list(range(8)), trace=trace
    )
    LAST_RESULTS = res

    out = np.empty((B, S, D), dtype=np.float32)
    for b in range(B):
        av0 = np.asarray(res.results[2 * b]["av"], dtype=np.float32)
        av1 = np.asarray(res.results[2 * b + 1]["av"], dtype=np.float32)
        den0 = res.results[2 * b]["den"]
        den1 = res.results[2 * b + 1]["den"]
        # odd-parity core's query columns are half-swapped: undo
        av = av0 + av1[:, swap]
        den = den0 + den1[:, swap]
        out[b] = (av / den).T
    return out
